# revision 6
# baseline (speedup 1.0000x reference)
"""Trainium2 Bass kernel for nn_ConvexSimilarityHash.

Reference computation (B=16, T=4096, E=1024, HALF=2048, WIN=15):
  x_t    = x * taper[None, :, None]
  c0     = x_t[..., 0];  r = |c0| + 1e-6
  start  = where(c0 >= 0, 0, pi)
  v      = clip(x_t[..., 1:] / r, -1+1e-6, 1-1e-6)
  phases = start + sum_e arcsin(v)                       # (B, T)
  tdraw  = causal triangular MA of c0 over WIN lags      # (B, T)
  thash  = tanh(silu(phases @ cW1.T + cb1) @ cW2.T + cb2)
  tdiff  = tanh(silu(tdraw  @ pW1.T + pb1) @ pW2.T + pb2)
  out    = stack([thash, tdiff], -1)                     # (B, HALF, 2)

Sharding (8 cores): T-sharded elementwise + layer-1 partial matmuls,
AllReduce of the (2, HALF, B) pre-activations, feature-sharded layer 2.

arcsin(w) = atan(w / sqrt(1 - w^2)), w = s*wc, s = taper/r:
  wc = clip(x, lo, hi)          DVE TensorScalarPtr (2 el/cyc)
  u  = (wc*ss)*wc = w^2         GPSIMD scalar_tensor_tensor
  q  = 1/sqrt(|1-u|)            ACT Abs_reciprocal_sqrt, scale=-1 bias=+1
                                (immediates shared by all b -> quad-fused)
  y  = wc*q  (fp16)             DVE TensorTensor
  ph += atan(s*y)               ACT Arctan, scale=s ptr, accum_out

Host precomputes per-(t,b) scalars lo/hi/ss/s/startT and the tiny tdraw
conv (B*T*WIN flops).  hi is nudged down so fl(fl(hi*ss)*hi) <= CLIP^2 in
fp32 -- boundary-clipped elements hit the rsqrt with 1-u >= 2e-6, no
separate clamp pass needed.

Arctan and Abs_reciprocal_sqrt live in different HW act-table sets; each
alternation costs a 1283ns table reload.  All atans of a chunk take their
(zero) bias from a column derived from the LAST quad rsqrt, so the ACT
queue batches 4 rsqrts then 16 atans per chunk (~2 loads/chunk, vs ~15
for the readiness-driven interleave).
"""
from contextlib import ExitStack

import numpy as np

import concourse.bacc as bacc
import concourse.tile as tile
from concourse import mybir
from concourse.bass_utils import run_bass_kernel_spmd

AF = mybir.ActivationFunctionType
ALU = mybir.AluOpType
F32 = mybir.dt.float32
F16 = mybir.dt.float16

B, T, E = 16, 4096, 1024
HALF = T // 2
WIN = 15
EPS = 1e-6
CLIP = 1.0 - 1e-6
CLIP2 = np.float32(np.float32(CLIP) * np.float32(CLIP))
NCORE = 8
TLOC = T // NCORE          # 512 timesteps per core
NCH = TLOC // 128          # 4 chunks of 128 timesteps
NQ = B // 4                # 4 quads of 4 batch rows
OSL = HALF // NCORE        # 256 output features per core
NOT = OSL // 128           # 2 output tiles per compressor
NHT = HALF // 128          # 16 hidden tiles
PI = float(np.pi)


def build_nc(no_cc=False, debug_taps=False, y_f32=False):
    nc = bacc.Bacc("TRN2", target_bir_lowering=False, debug=False,
                   num_devices=NCORE)

    xs_h = nc.dram_tensor("xs", [B, TLOC, E], F32, kind="ExternalInput")
    lo_h = nc.dram_tensor("lo", [128, NCH * B], F32, kind="ExternalInput")
    hi_h = nc.dram_tensor("hi", [128, NCH * B], F32, kind="ExternalInput")
    ss_h = nc.dram_tensor("ss", [128, NCH * B], F32, kind="ExternalInput")
    sT_h = nc.dram_tensor("sT", [128, NCH * B], F32, kind="ExternalInput")
    st_h = nc.dram_tensor("st", [128, NCH * B], F32, kind="ExternalInput")
    td_h = nc.dram_tensor("td", [128, NCH * B], F32, kind="ExternalInput")
    w1c_h = nc.dram_tensor("w1c", [TLOC, HALF], F32, kind="ExternalInput")
    w1p_h = nc.dram_tensor("w1p", [TLOC, HALF], F32, kind="ExternalInput")
    w2c_h = nc.dram_tensor("w2c", [HALF, OSL], F32, kind="ExternalInput")
    w2p_h = nc.dram_tensor("w2p", [HALF, OSL], F32, kind="ExternalInput")
    b1big_h = nc.dram_tensor("b1big", [128, 2 * NHT * B], F32, kind="ExternalInput")
    b2c_h = nc.dram_tensor("b2c", [128, 2 * NOT], F32, kind="ExternalInput")
    out_h = nc.dram_tensor("out", [2, NOT, 128, B], F32, kind="ExternalOutput")

    cc_in = nc.dram_tensor("cc_in", [128, 2 * NHT * B], F32)
    cc_out = nc.dram_tensor("cc_out", [128, 2 * NHT * B], F32, addr_space="Shared")

    dbg = {}
    if debug_taps:
        for nm, shp in (("dbg_ph", [128, NCH * B]),
                        ("dbg_pre", [128, 2 * NHT * B])):
            dbg[nm] = nc.dram_tensor(nm, shp, F32, kind="ExternalOutput")

    YDT = F32 if y_f32 else F16

    with tile.TileContext(nc) as tc, ExitStack() as ctx:
        consts = ctx.enter_context(tc.tile_pool(name="consts", bufs=1))
        w1pool = ctx.enter_context(tc.tile_pool(name="w1", bufs=2))
        w2pool = ctx.enter_context(tc.tile_pool(name="w2", bufs=1))
        xpool = ctx.enter_context(tc.tile_pool(name="x", bufs=10))
        uqp = ctx.enter_context(tc.tile_pool(name="uq", bufs=1))
        ypool = ctx.enter_context(tc.tile_pool(name="y", bufs=1))
        small = ctx.enter_context(tc.tile_pool(name="small", bufs=2))
        tailp = ctx.enter_context(tc.tile_pool(name="tail", bufs=1))
        psum = ctx.enter_context(tc.tile_pool(name="psum", bufs=2, space="PSUM"))
        ps2 = ctx.enter_context(tc.tile_pool(name="ps2", bufs=2, space="PSUM"))

        def load_const(h, shape):
            t = consts.tile(shape, F32, tag=h.name)
            nc.sync.dma_start(out=t, in_=h.ap())
            return t

        lo = load_const(lo_h, [128, NCH * B])
        hi = load_const(hi_h, [128, NCH * B])
        ss = load_const(ss_h, [128, NCH * B])
        sT = load_const(sT_h, [128, NCH * B])
        startT = load_const(st_h, [128, NCH * B])
        tdraw = load_const(td_h, [128, NCH * B])
        b1big = load_const(b1big_h, [128, 2 * NHT * B])
        b2c = load_const(b2c_h, [128, 2 * NOT])

        # SBUF accumulator for layer-1 pre-activations
        sacc = consts.tile([128, 2 * NHT * B], F32, tag="sacc")

        xs = xs_h.ap()
        w2t = {}

        for j in range(NCH):
            cs = slice(j * B, (j + 1) * B)

            wcts = []
            qts = []
            for q in range(NQ):
                uq = uqp.tile([128, 4 * E], F32, tag=f"uq{q % 2}")
                for b4 in range(4):
                    b = q * 4 + b4
                    col = j * B + b
                    xt = xpool.tile([128, E], F32, tag="xt")
                    nc.sync.dma_start(out=xt, in_=xs[b, j * 128:(j + 1) * 128, :])
                    # in-place clip: wc = min(max(x, lo), hi)
                    nc.vector.tensor_scalar(out=xt, in0=xt,
                                            scalar1=lo[:, col:col + 1],
                                            scalar2=hi[:, col:col + 1],
                                            op0=ALU.max, op1=ALU.min)
                    # u = (wc*ss)*wc = w^2 (STT not supported on GPSIMD HW)
                    nc.vector.scalar_tensor_tensor(
                        out=uq[:, b4 * E:(b4 + 1) * E], in0=xt,
                        scalar=ss[:, col:col + 1], in1=xt,
                        op0=ALU.mult, op1=ALU.mult)
                    wcts.append(xt)
                # q = 1/sqrt(|1-u|), quad-fused, in place
                nc.scalar.activation(out=uq, in_=uq, func=AF.Abs_reciprocal_sqrt,
                                     scale=-1.0, bias=1.0)
                qts.append(uq)

            # gate: all chunk atans take bias (=0) derived from the LAST
            # quad rsqrt, so ACT batches rsqrts then atans (2 table loads)
            zcol = small.tile([128, 1], F32, tag="zcol")
            nc.vector.tensor_scalar(out=zcol, in0=qts[-1][:, 0:1], scalar1=0.0,
                                    scalar2=None, op0=ALU.mult)

            ytiles = []
            for q in range(NQ):
                uq = qts[q]
                for b4 in range(4):
                    b = q * 4 + b4
                    yt = ypool.tile([128, E], YDT, tag=f"y{b}")
                    # y = wc*q on GPSIMD to offload DVE
                    nc.gpsimd.tensor_tensor(out=yt, in0=wcts[b],
                                            in1=uq[:, b4 * E:(b4 + 1) * E],
                                            op=ALU.mult)
                    ytiles.append(yt)

            phacc = small.tile([128, B], F32, tag="phacc")
            for b in range(B):
                col = j * B + b
                nc.scalar.activation(out=ytiles[b][:, 1:E], in_=ytiles[b][:, 1:E],
                                     func=AF.Arctan, scale=sT[:, col:col + 1],
                                     bias=zcol[:, 0:1],
                                     accum_out=phacc[:, b:b + 1])

            phasesT = small.tile([128, B], F32, tag="phasesT")
            nc.vector.tensor_tensor(out=phasesT, in0=phacc,
                                    in1=startT[:, cs], op=ALU.add)

            if debug_taps:
                nc.sync.dma_start(out=dbg["dbg_ph"].ap()[:, cs], in_=phasesT)

            # layer-1 partial matmuls: single-shot groups into one PSUM tile
            pl1 = psum.tile([128, 2 * NHT * B], F32, tag="pl1")
            for c, (w1h, rhs) in enumerate(((w1c_h, phasesT),
                                            (w1p_h, tdraw[:, cs]))):
                w1tile = w1pool.tile([128, HALF], F32, tag=f"w1_{c}")
                nc.sync.dma_start(out=w1tile,
                                  in_=w1h.ap()[j * 128:(j + 1) * 128, :])
                for ht in range(NHT):
                    nc.tensor.matmul(
                        pl1[:, (c * NHT + ht) * B:(c * NHT + ht + 1) * B],
                        lhsT=w1tile[:, ht * 128:(ht + 1) * 128],
                        rhs=rhs, start=True, stop=True,
                        skip_group_check=True)
            if j == 0:
                nc.vector.tensor_copy(out=sacc, in_=pl1)
            else:
                nc.vector.tensor_tensor(out=sacc, in0=sacc, in1=pl1, op=ALU.add)

            if j == NCH - 2:
                # prefetch layer-2 weights late (keeps SBUF headroom earlier)
                for c, w2h in ((0, w2c_h), (1, w2p_h)):
                    for kk in range(NHT):
                        t = w2pool.tile([128, OSL], F32, tag=f"w2_{c}_{kk}")
                        nc.sync.dma_start(
                            out=t, in_=w2h.ap()[kk * 128:(kk + 1) * 128, :])
                        w2t[(c, kk)] = t

        # ---- tail: bias, AllReduce, silu, layer 2, tanh ----
        pre = tailp.tile([128, 2 * NHT * B], F32, tag="pre")
        nc.vector.tensor_tensor(out=pre, in0=sacc, in1=b1big, op=ALU.add)
        nc.sync.dma_start(out=cc_in.ap(), in_=pre)
        red = tailp.tile([128, 2 * NHT * B], F32, tag="red")
        if no_cc:
            # timing-sim variant: skip the collective, keep equivalent DMAs
            nc.sync.dma_start(out=red, in_=cc_in.ap())
        else:
            nc.gpsimd.collective_compute(
                "AllReduce", ALU.add, replica_groups=[list(range(NCORE))],
                ins=[cc_in.ap()], outs=[cc_out.ap()])
            nc.sync.dma_start(out=red, in_=cc_out.ap())
        h1 = tailp.tile([128, 2 * NHT * B], F32, tag="h1")
        nc.scalar.activation(out=h1, in_=red, func=AF.Silu)

        if debug_taps:
            nc.sync.dma_start(out=dbg["dbg_pre"].ap(), in_=pre)

        for c in range(2):
            for ot in range(NOT):
                p2 = ps2.tile([128, B], F32, tag="p2")
                for kk in range(NHT):
                    nc.tensor.matmul(
                        p2, lhsT=w2t[(c, kk)][:, ot * 128:(ot + 1) * 128],
                        rhs=h1[:, (c * NHT + kk) * B:(c * NHT + kk + 1) * B],
                        start=(kk == 0), stop=(kk == NHT - 1))
                ot_sb = small.tile([128, B], F32, tag="ot_sb")
                nc.scalar.activation(out=ot_sb, in_=p2, func=AF.Tanh,
                                     bias=b2c[:, (c * NOT + ot):(c * NOT + ot) + 1])
                nc.sync.dma_start(out=out_h.ap()[c, ot, :, :], in_=ot_sb)

    nc.compile()
    return nc


def host_prepare(x, taper, cW1, cb1, cW2, cb2, pW1, pb1, pW2, pb2):
    """Build the 8 per-core input maps (numpy only)."""
    x = np.ascontiguousarray(np.asarray(x), dtype=np.float32)
    taper = np.asarray(taper, dtype=np.float32)
    cW1, cW2, pW1, pW2 = (np.asarray(a, np.float32) for a in (cW1, cW2, pW1, pW2))
    cb1, cb2, pb1, pb2 = (np.asarray(a, np.float32) for a in (cb1, cb2, pb1, pb2))

    c0 = (x[:, :, 0] * taper[None, :]).astype(np.float32)       # (B, T)
    r = (np.abs(c0) + np.float32(EPS)).astype(np.float32)
    s64 = taper[None, :].astype(np.float64) / r.astype(np.float64)
    sF = s64.astype(np.float32)                                  # (B, T)
    ssF = (s64 * s64).astype(np.float32)
    # hi: largest fp32 with fl(fl(hi*ss)*hi) <= CLIP2 (device STT order)
    with np.errstate(divide="ignore", invalid="ignore"):
        hi0 = np.sqrt(np.float64(CLIP2) / (s64 * s64))
    hiF = hi0.astype(np.float32)
    zs = ssF == 0.0
    hiF[zs] = 1.0
    for _ in range(4):
        u = ((hiF * ssF).astype(np.float32) * hiF).astype(np.float32)
        bad = u > CLIP2
        if not bad.any():
            break
        hiF[bad] = np.nextafter(hiF[bad], np.float32(0.0), dtype=np.float32)
    loF = (-hiF).astype(np.float32)
    startF = np.where(c0 >= 0, np.float32(0.0), np.float32(PI)).astype(np.float32)

    # tdraw: causal triangular MA + compressor input scaling (host, B*T*WIN)
    xp = np.concatenate([np.zeros((B, WIN), np.float32), c0], axis=1)
    num = np.zeros_like(c0)
    for d in range(1, WIN + 1):
        num = num + np.float32(d) * xp[:, WIN - d:WIN - d + T]
    i = np.arange(T, dtype=np.float32)
    sN = np.minimum(i, np.float32(WIN))
    norm = sN * (sN + 1.0) * 0.5
    tdF = np.where(norm > 0, num / np.maximum(norm, 1.0), 0.0).astype(np.float32)

    in_maps = []
    for k in range(NCORE):
        t0 = k * TLOC
        tsl = slice(t0, t0 + TLOC)
        xs = np.ascontiguousarray(x[:, tsl, :])

        def cc(a):
            # (B, TLOC) -> [128, NCH*B], column = j*B + b
            blk = a[:, tsl].reshape(B, NCH, 128)
            return np.ascontiguousarray(
                blk.transpose(2, 1, 0).reshape(128, NCH * B))

        w1c = np.ascontiguousarray(cW1[:, tsl].T)
        w1p = np.ascontiguousarray(pW1[:, tsl].T)
        osl = slice(k * OSL, (k + 1) * OSL)
        w2c = np.ascontiguousarray(cW2[osl, :].T)
        w2p = np.ascontiguousarray(pW2[osl, :].T)

        b1big = np.empty((128, 2, NHT, B), np.float32)
        for c, b1 in enumerate((cb1, pb1)):
            b1m = b1.reshape(NHT, 128).T                      # [p, ht]
            b1big[:, c, :, :] = b1m[:, :, None]
        # each core adds the bias before the AllReduce -> divide by NCORE
        b1big = np.ascontiguousarray(b1big.reshape(128, -1)) / np.float32(NCORE)

        b2cols = np.empty((128, 2, NOT), np.float32)
        for c, b2 in enumerate((cb2, pb2)):
            b2cols[:, c, :] = b2[osl].reshape(NOT, 128).T
        b2cols = np.ascontiguousarray(b2cols.reshape(128, -1))

        in_maps.append(dict(
            xs=xs, lo=cc(loF), hi=cc(hiF), ss=cc(ssF), sT=cc(sF),
            st=cc(startF), td=cc(tdF),
            w1c=w1c, w1p=w1p, w2c=w2c, w2p=w2p,
            b1big=b1big, b2c=b2cols))
    return in_maps


def assemble_output(results):
    out = np.empty((B, HALF, 2), np.float32)
    for k, r in enumerate(results):
        o = np.asarray(r["out"]).reshape(2, NOT, 128, B)      # [c, ot, p, b]
        for c in range(2):
            for ot in range(NOT):
                out[:, k * OSL + ot * 128:k * OSL + (ot + 1) * 128, c] = o[c, ot].T
    return out


_NC_CACHE = {}


def _get_nc(**kw):
    key = tuple(sorted(kw.items()))
    if key not in _NC_CACHE:
        _NC_CACHE[key] = build_nc(**kw)
    return _NC_CACHE[key]


def run(inputs, trace=False, **build_kw):
    nc = _get_nc(**build_kw)
    in_maps = host_prepare(**inputs)
    res = run_bass_kernel_spmd(nc, in_maps, core_ids=list(range(NCORE)),
                               trace=trace)
    return assemble_output(res.results), res


def kernel(**inputs):
    out, _ = run(inputs)
    return out


# revision 14
# speedup vs baseline: 1.2671x; 1.2671x over previous
"""Trainium2 Bass kernel for nn_ConvexSimilarityHash.

Reference computation (B=16, T=4096, E=1024, HALF=2048, WIN=15):
  x_t    = x * taper[None, :, None]
  c0     = x_t[..., 0];  r = |c0| + 1e-6
  start  = where(c0 >= 0, 0, pi)
  v      = clip(x_t[..., 1:] / r, -1+1e-6, 1-1e-6)
  phases = start + sum_e arcsin(v)                       # (B, T)
  tdraw  = causal triangular MA of c0 over WIN lags      # (B, T)
  thash  = tanh(silu(phases @ cW1.T + cb1) @ cW2.T + cb2)
  tdiff  = tanh(silu(tdraw  @ pW1.T + pb1) @ pW2.T + pb2)
  out    = stack([thash, tdiff], -1)                     # (B, HALF, 2)

Sharding (8 cores): T-sharded elementwise + layer-1 partial matmuls,
AllReduce of the (2, HALF, B) pre-activations, feature-sharded layer 2.

arcsin(w) = atan(w / sqrt(1 - w^2)), w = s*wc, s = taper/r:
  wc = clip(x, lo, hi)          DVE TensorScalarPtr (2 el/cyc)
  u  = (wc*ss)*wc = w^2         GPSIMD scalar_tensor_tensor
  q  = 1/sqrt(|1-u|)            ACT Abs_reciprocal_sqrt, scale=-1 bias=+1
                                (immediates shared by all b -> quad-fused)
  y  = wc*q  (fp16)             DVE TensorTensor
  ph += atan(s*y)               ACT Arctan, scale=s ptr, accum_out

Host precomputes per-(t,b) scalars lo/hi/ss/s/startT and the tiny tdraw
conv (B*T*WIN flops).  hi is nudged down so fl(fl(hi*ss)*hi) <= CLIP^2 in
fp32 -- boundary-clipped elements hit the rsqrt with 1-u >= 2e-6, no
separate clamp pass needed.

Arctan and Abs_reciprocal_sqrt live in different HW act-table sets; each
alternation costs a 1283ns table reload.  All atans of a chunk take their
(zero) bias from a column derived from the LAST quad rsqrt, so the ACT
queue batches 4 rsqrts then 16 atans per chunk (~2 loads/chunk, vs ~15
for the readiness-driven interleave).
"""
from contextlib import ExitStack

import numpy as np

import concourse.bacc as bacc
import concourse.tile as tile
from concourse import mybir
from concourse.bass_utils import run_bass_kernel_spmd

AF = mybir.ActivationFunctionType
ALU = mybir.AluOpType
F32 = mybir.dt.float32
F16 = mybir.dt.float16

B, T, E = 16, 4096, 1024
HALF = T // 2
WIN = 15
EPS = 1e-6
CLIP = 1.0 - 1e-6
CLIP2 = np.float32(np.float32(CLIP) * np.float32(CLIP))
NCORE = 8
TLOC = T // NCORE          # 512 timesteps per core
NCH = TLOC // 128          # 4 chunks of 128 timesteps
NQ = B // 4                # 4 quads of 4 batch rows
OSL = HALF // NCORE        # 256 output features per core
NOT = OSL // 128           # 2 output tiles per compressor
NHT = HALF // 128          # 16 hidden tiles
PI = float(np.pi)


def build_nc(no_cc=False, debug_taps=False, y_f32=False):
    nc = bacc.Bacc("TRN2", target_bir_lowering=False, debug=False,
                   num_devices=NCORE)

    xs_h = nc.dram_tensor("xs", [B, TLOC, E], F32, kind="ExternalInput")
    lo_h = nc.dram_tensor("lo", [128, NCH * B], F32, kind="ExternalInput")
    hi_h = nc.dram_tensor("hi", [128, NCH * B], F32, kind="ExternalInput")
    ss_h = nc.dram_tensor("ss", [128, NCH * B], F32, kind="ExternalInput")
    sT_h = nc.dram_tensor("sT", [128, NCH * B], F32, kind="ExternalInput")
    st_h = nc.dram_tensor("st", [128, NCH * B], F32, kind="ExternalInput")
    td_h = nc.dram_tensor("td", [128, NCH * B], F32, kind="ExternalInput")
    w1c_h = nc.dram_tensor("w1c", [TLOC, HALF], F32, kind="ExternalInput")
    w1p_h = nc.dram_tensor("w1p", [TLOC, HALF], F32, kind="ExternalInput")
    w2c_h = nc.dram_tensor("w2c", [HALF, OSL], F16, kind="ExternalInput")
    w2p_h = nc.dram_tensor("w2p", [HALF, OSL], F16, kind="ExternalInput")
    b1big_h = nc.dram_tensor("b1big", [128, 2 * NHT * B], F32, kind="ExternalInput")
    b2c_h = nc.dram_tensor("b2c", [128, 2 * NOT], F32, kind="ExternalInput")
    out_h = nc.dram_tensor("out", [2, NOT, 128, B], F32, kind="ExternalOutput")

    cc_in = nc.dram_tensor("cc_in", [128, 2 * NHT * B], F32)
    cc_out = nc.dram_tensor("cc_out", [128, 2 * NHT * B], F32, addr_space="Shared")

    dbg = {}
    if debug_taps:
        for nm, shp in (("dbg_ph", [128, NCH * B]),
                        ("dbg_pre", [128, 2 * NHT * B])):
            dbg[nm] = nc.dram_tensor(nm, shp, F32, kind="ExternalOutput")

    YDT = F32 if y_f32 else F16

    with tile.TileContext(nc) as tc, ExitStack() as ctx:
        consts = ctx.enter_context(tc.tile_pool(name="consts", bufs=1))
        w1pool = ctx.enter_context(tc.tile_pool(name="w1", bufs=1))
        w2pool = ctx.enter_context(tc.tile_pool(name="w2", bufs=1))
        xpool = ctx.enter_context(tc.tile_pool(name="x", bufs=16))
        uqp = ctx.enter_context(tc.tile_pool(name="uq", bufs=1))
        ypool = ctx.enter_context(tc.tile_pool(name="y", bufs=1))
        small = ctx.enter_context(tc.tile_pool(name="small", bufs=2))
        tailp = ctx.enter_context(tc.tile_pool(name="tail", bufs=1))
        psum = ctx.enter_context(tc.tile_pool(name="psum", bufs=2, space="PSUM"))
        ps2 = ctx.enter_context(tc.tile_pool(name="ps2", bufs=2, space="PSUM"))

        def load_const(h, shape):
            t = consts.tile(shape, F32, tag=h.name)
            nc.sync.dma_start(out=t, in_=h.ap())
            return t

        lo = load_const(lo_h, [128, NCH * B])
        hi = load_const(hi_h, [128, NCH * B])
        ss = load_const(ss_h, [128, NCH * B])
        sT = load_const(sT_h, [128, NCH * B])
        startT = load_const(st_h, [128, NCH * B])
        tdraw = load_const(td_h, [128, NCH * B])
        b1big = load_const(b1big_h, [128, 2 * NHT * B])
        b2c = load_const(b2c_h, [128, 2 * NOT])

        # SBUF accumulator for layer-1 pre-activations
        sacc = consts.tile([128, 2 * NHT * B], F32, tag="sacc")

        xs = xs_h.ap()
        w2t = {}

        for j in range(NCH):
            cs = slice(j * B, (j + 1) * B)

            wcts = []
            qts = []
            for q in range(NQ):
                uq = uqp.tile([128, 4 * E], F32, tag=f"uq{q}")
                for b4 in range(4):
                    b = q * 4 + b4
                    col = j * B + b
                    xt = xpool.tile([128, E], F32, tag="xt")
                    nc.sync.dma_start(out=xt, in_=xs[b, j * 128:(j + 1) * 128, :])
                    # in-place clip: wc = min(max(x, lo), hi)
                    nc.vector.tensor_scalar(out=xt, in0=xt,
                                            scalar1=lo[:, col:col + 1],
                                            scalar2=hi[:, col:col + 1],
                                            op0=ALU.max, op1=ALU.min)
                    # u = (wc*ss)*wc = w^2 (STT not supported on GPSIMD HW)
                    nc.vector.scalar_tensor_tensor(
                        out=uq[:, b4 * E:(b4 + 1) * E], in0=xt,
                        scalar=ss[:, col:col + 1], in1=xt,
                        op0=ALU.mult, op1=ALU.mult)
                    wcts.append(xt)
                # q = 1/sqrt(|1-u|), quad-fused, in place
                nc.scalar.activation(out=uq, in_=uq, func=AF.Abs_reciprocal_sqrt,
                                     scale=-1.0, bias=1.0)
                qts.append(uq)

            # gate: all chunk atans take bias (=0) derived from the LAST
            # quad rsqrt, so ACT batches rsqrts then atans (2 table loads)
            zcol = small.tile([128, 1], F32, tag="zcol")
            nc.vector.tensor_scalar(out=zcol, in0=qts[-1][:, 0:1], scalar1=0.0,
                                    scalar2=None, op0=ALU.mult)

            ytiles = []
            for q in range(NQ):
                uq = qts[q]
                for b4 in range(4):
                    b = q * 4 + b4
                    yt = ypool.tile([128, E], YDT, tag=f"y{b}")
                    # y = wc*q on GPSIMD to offload DVE; the last two go on
                    # DVE so the atan batch isn't stalled by the Pool tail
                    eng = nc.vector if b >= B - 2 else nc.gpsimd
                    eng.tensor_tensor(out=yt, in0=wcts[b],
                                      in1=uq[:, b4 * E:(b4 + 1) * E],
                                      op=ALU.mult)
                    ytiles.append(yt)

            phacc = small.tile([128, B], F32, tag="phacc")
            for b in range(B):
                col = j * B + b
                nc.scalar.activation(out=ytiles[b][:, 1:E], in_=ytiles[b][:, 1:E],
                                     func=AF.Arctan, scale=sT[:, col:col + 1],
                                     bias=zcol[:, 0:1],
                                     accum_out=phacc[:, b:b + 1])

            phasesT = small.tile([128, B], F32, tag="phasesT")
            nc.vector.tensor_tensor(out=phasesT, in0=phacc,
                                    in1=startT[:, cs], op=ALU.add)

            if debug_taps:
                nc.sync.dma_start(out=dbg["dbg_ph"].ap()[:, cs], in_=phasesT)

            # layer-1 partial matmuls: single-shot groups into one PSUM tile
            pl1 = psum.tile([128, 2 * NHT * B], F32, tag="pl1")
            for c, (w1h, rhs) in enumerate(((w1c_h, phasesT),
                                            (w1p_h, tdraw[:, cs]))):
                w1tile = w1pool.tile([128, HALF], F32, tag=f"w1_{c}")
                nc.sync.dma_start(out=w1tile,
                                  in_=w1h.ap()[j * 128:(j + 1) * 128, :])
                for ht in range(NHT):
                    nc.tensor.matmul(
                        pl1[:, (c * NHT + ht) * B:(c * NHT + ht + 1) * B],
                        lhsT=w1tile[:, ht * 128:(ht + 1) * 128],
                        rhs=rhs, start=True, stop=True,
                        skip_group_check=True)
            if j == 0:
                nc.vector.tensor_copy(out=sacc, in_=pl1)
            else:
                nc.vector.tensor_tensor(out=sacc, in0=sacc, in1=pl1, op=ALU.add)

            if j == NCH - 2:
                # prefetch layer-2 weights late (keeps SBUF headroom earlier)
                for c, w2h in ((0, w2c_h), (1, w2p_h)):
                    for kk in range(NHT):
                        t = w2pool.tile([128, OSL], F16, tag=f"w2_{c}_{kk}")
                        nc.sync.dma_start(
                            out=t, in_=w2h.ap()[kk * 128:(kk + 1) * 128, :])
                        w2t[(c, kk)] = t

        # ---- tail: bias, AllReduce, silu, layer 2, tanh ----
        pre = tailp.tile([128, 2 * NHT * B], F32, tag="pre")
        nc.vector.tensor_tensor(out=pre, in0=sacc, in1=b1big, op=ALU.add)
        nc.sync.dma_start(out=cc_in.ap(), in_=pre)
        red = tailp.tile([128, 2 * NHT * B], F32, tag="red")
        if no_cc:
            # timing-sim variant: skip the collective, keep equivalent DMAs
            nc.sync.dma_start(out=red, in_=cc_in.ap())
        else:
            nc.gpsimd.collective_compute(
                "AllReduce", ALU.add, replica_groups=[list(range(NCORE))],
                ins=[cc_in.ap()], outs=[cc_out.ap()])
            nc.sync.dma_start(out=red, in_=cc_out.ap())
        h1 = tailp.tile([128, 2 * NHT * B], F16, tag="h1")
        nc.scalar.activation(out=h1, in_=red, func=AF.Silu)

        if debug_taps:
            nc.sync.dma_start(out=dbg["dbg_pre"].ap(), in_=pre)

        for c in range(2):
            for ot in range(NOT):
                p2 = ps2.tile([128, B], F32, tag="p2")
                for kk in range(NHT):
                    nc.tensor.matmul(
                        p2, lhsT=w2t[(c, kk)][:, ot * 128:(ot + 1) * 128],
                        rhs=h1[:, (c * NHT + kk) * B:(c * NHT + kk + 1) * B],
                        start=(kk == 0), stop=(kk == NHT - 1))
                ot_sb = small.tile([128, B], F32, tag="ot_sb")
                nc.scalar.activation(out=ot_sb, in_=p2, func=AF.Tanh,
                                     bias=b2c[:, (c * NOT + ot):(c * NOT + ot) + 1])
                nc.sync.dma_start(out=out_h.ap()[c, ot, :, :], in_=ot_sb)

    nc.compile()
    return nc


def host_prepare(x, taper, cW1, cb1, cW2, cb2, pW1, pb1, pW2, pb2):
    """Build the 8 per-core input maps (numpy only)."""
    x = np.ascontiguousarray(np.asarray(x), dtype=np.float32)
    taper = np.asarray(taper, dtype=np.float32)
    cW1, cW2, pW1, pW2 = (np.asarray(a, np.float32) for a in (cW1, cW2, pW1, pW2))
    cb1, cb2, pb1, pb2 = (np.asarray(a, np.float32) for a in (cb1, cb2, pb1, pb2))

    c0 = (x[:, :, 0] * taper[None, :]).astype(np.float32)       # (B, T)
    r = (np.abs(c0) + np.float32(EPS)).astype(np.float32)
    s64 = taper[None, :].astype(np.float64) / r.astype(np.float64)
    sF = s64.astype(np.float32)                                  # (B, T)
    ssF = (s64 * s64).astype(np.float32)
    # hi: largest fp32 with fl(fl(hi*ss)*hi) <= CLIP2 (device STT order)
    with np.errstate(divide="ignore", invalid="ignore"):
        hi0 = np.sqrt(np.float64(CLIP2) / (s64 * s64))
    hiF = hi0.astype(np.float32)
    zs = ssF == 0.0
    hiF[zs] = 1.0
    for _ in range(4):
        u = ((hiF * ssF).astype(np.float32) * hiF).astype(np.float32)
        bad = u > CLIP2
        if not bad.any():
            break
        hiF[bad] = np.nextafter(hiF[bad], np.float32(0.0), dtype=np.float32)
    loF = (-hiF).astype(np.float32)
    startF = np.where(c0 >= 0, np.float32(0.0), np.float32(PI)).astype(np.float32)

    # tdraw: causal triangular MA + compressor input scaling (host, B*T*WIN)
    xp = np.concatenate([np.zeros((B, WIN), np.float32), c0], axis=1)
    num = np.zeros_like(c0)
    for d in range(1, WIN + 1):
        num = num + np.float32(d) * xp[:, WIN - d:WIN - d + T]
    i = np.arange(T, dtype=np.float32)
    sN = np.minimum(i, np.float32(WIN))
    norm = sN * (sN + 1.0) * 0.5
    tdF = np.where(norm > 0, num / np.maximum(norm, 1.0), 0.0).astype(np.float32)

    in_maps = []
    for k in range(NCORE):
        t0 = k * TLOC
        tsl = slice(t0, t0 + TLOC)
        xs = np.ascontiguousarray(x[:, tsl, :])

        def cc(a):
            # (B, TLOC) -> [128, NCH*B], column = j*B + b
            blk = a[:, tsl].reshape(B, NCH, 128)
            return np.ascontiguousarray(
                blk.transpose(2, 1, 0).reshape(128, NCH * B))

        w1c = np.ascontiguousarray(cW1[:, tsl].T)
        w1p = np.ascontiguousarray(pW1[:, tsl].T)
        osl = slice(k * OSL, (k + 1) * OSL)
        w2c = np.ascontiguousarray(cW2[osl, :].T.astype(np.float16))
        w2p = np.ascontiguousarray(pW2[osl, :].T.astype(np.float16))

        b1big = np.empty((128, 2, NHT, B), np.float32)
        for c, b1 in enumerate((cb1, pb1)):
            b1m = b1.reshape(NHT, 128).T                      # [p, ht]
            b1big[:, c, :, :] = b1m[:, :, None]
        # each core adds the bias before the AllReduce -> divide by NCORE
        b1big = np.ascontiguousarray(b1big.reshape(128, -1)) / np.float32(NCORE)

        b2cols = np.empty((128, 2, NOT), np.float32)
        for c, b2 in enumerate((cb2, pb2)):
            b2cols[:, c, :] = b2[osl].reshape(NOT, 128).T
        b2cols = np.ascontiguousarray(b2cols.reshape(128, -1))

        in_maps.append(dict(
            xs=xs, lo=cc(loF), hi=cc(hiF), ss=cc(ssF), sT=cc(sF),
            st=cc(startF), td=cc(tdF),
            w1c=w1c, w1p=w1p, w2c=w2c, w2p=w2p,
            b1big=b1big, b2c=b2cols))
    return in_maps


def assemble_output(results):
    out = np.empty((B, HALF, 2), np.float32)
    for k, r in enumerate(results):
        o = np.asarray(r["out"]).reshape(2, NOT, 128, B)      # [c, ot, p, b]
        for c in range(2):
            for ot in range(NOT):
                out[:, k * OSL + ot * 128:k * OSL + (ot + 1) * 128, c] = o[c, ot].T
    return out


_NC_CACHE = {}


def _get_nc(**kw):
    key = tuple(sorted(kw.items()))
    if key not in _NC_CACHE:
        _NC_CACHE[key] = build_nc(**kw)
    return _NC_CACHE[key]


def run(inputs, trace=False, **build_kw):
    nc = _get_nc(**build_kw)
    in_maps = host_prepare(**inputs)
    res = run_bass_kernel_spmd(nc, in_maps, core_ids=list(range(NCORE)),
                               trace=trace)
    return assemble_output(res.results), res


def kernel(**inputs):
    out, _ = run(inputs)
    return out
